# revision 1
# baseline (speedup 1.0000x reference)
"""Trainium2 Bass kernel for nn_DiagSSMBlock.

Math: s = x @ B  (T=4096, H=2048); h_t = a * h_{t-1} + s_t per channel
(equivalent to the reference depthwise causal conv with kernel a^t, since
|a| <= sqrt(2/H) ~= 0.031 the kernel decays below fp32 denormals within
~16 taps).  Output: (1, T, H).

Sharding: data-parallel over T across 8 cores; each core computes 512
timesteps (plus W=16 warm-up rows to rebuild the scan carry, exact to
fp32: a^17 ~= 2.6e-26).  Every core streams the full B.

Per-core device pipeline:
  - x chunk is pre-transposed on the host (sharding layout prep) into
    xT[p, k, t] = x[t, 128k + p], so the GEMM contraction dim lands on
    SBUF partitions with no on-device transpose.
  - GEMM: for each of 16 output-channel tiles m, accumulate 16 k-tile
    matmuls into PSUM (fp32r, moving free dim 264 >= 256 -> full PE rate).
  - Scan: tensor_tensor_scan (DVE) state = a*state + s straight out of
    PSUM into SBUF, chained across the two 264-wide chunks.
  - Output stays channel-major (h^T) on device; the host unshard
    restores (T, H) layout while gathering the 8 T-chunks.
"""

from contextlib import ExitStack

import numpy as np

T_FULL, H = 4096, 2048
N_CORES = 8
T_CHUNK = T_FULL // N_CORES  # 512
W = 16  # scan warm-up rows
T_SPAN = T_CHUNK + W  # 528
HALF = T_SPAN // 2  # 264 (>= 256 keeps fp32r matmul at full rate)
KT = H // 128  # 16 contraction tiles
MT = H // 128  # 16 output-channel tiles
XP = 16  # xT arrives as 16 single-slab pieces (fine-grained sems)

_CACHE = {}


def _build():
    import concourse.mybir as mybir
    import concourse.tile as tile
    from concourse import bacc

    f32 = mybir.dt.float32
    f32r = mybir.dt.float32r

    nc = bacc.Bacc("TRN2", target_bir_lowering=False, debug=False, num_devices=N_CORES)
    xT = nc.dram_tensor("xT", [128, KT, T_SPAN], f32r, kind="ExternalInput").ap()
    Bm = nc.dram_tensor("Bm", [MT, 128, KT, 128], f32r, kind="ExternalInput").ap()
    a = nc.dram_tensor("a", [128, MT], f32, kind="ExternalInput").ap()
    out = nc.dram_tensor("out", [MT, 128, T_CHUNK], f32, kind="ExternalOutput").ap()

    with tile.TileContext(nc) as tc, ExitStack() as ctx:
        const = ctx.enter_context(tc.tile_pool(name="const", bufs=1))
        xt_pool = ctx.enter_context(tc.tile_pool(name="xt", bufs=XP))
        b_pool = ctx.enter_context(tc.tile_pool(name="bm", bufs=8))
        ht_pool = ctx.enter_context(tc.tile_pool(name="ht", bufs=3))
        ps_gemm = ctx.enter_context(tc.tile_pool(name="psg", bufs=8, space="PSUM"))

        a_sb = const.tile([128, MT], f32)
        nc.sync.dma_start(out=a_sb, in_=a)

        rings = [nc.sync, nc.scalar]

        bms = {}
        xts = [None] * XP

        def load_xt(q2, ring):
            # two single-slab DMAs in the same FIFO slot the former
            # 2-slab piece occupied: byte-identical ring flow, but the
            # first slab's consumers unblock one slab earlier
            for k in (2 * q2, 2 * q2 + 1):
                t = xt_pool.tile([128, T_SPAN], f32r, tag="xt", name=f"xt{k}")
                ring.dma_start(out=t[:], in_=xT[:, k, :])
                xts[k] = t

        def load_bm_part(m, lo, hi, ring):
            if m not in bms:
                bms[m] = b_pool.tile([128, KT * 128], f32r, tag="bm", name=f"bm{m}")
            ring.dma_start(
                out=bms[m][:, lo * 128 : hi * 128].rearrange(
                    "p (k c) -> p k c", k=hi - lo
                ),
                in_=Bm[m, :, lo:hi, :],
            )

        # Ring FIFO plan: first matmul needs bm0[k0:2] + xt piece 0; B
        # slabs for m=1..3 drip in between xT pieces so each m-tile can
        # join the phase-1 interleave shortly after the previous.
        load_bm_part(0, 0, 2, nc.sync)
        load_xt(0, nc.scalar)
        load_bm_part(0, 2, 8, nc.sync)
        load_bm_part(0, 8, 16, nc.scalar)
        load_xt(1, nc.sync)
        load_xt(2, nc.scalar)
        load_bm_part(1, 0, 8, nc.sync)
        load_bm_part(1, 8, 16, nc.scalar)
        load_xt(3, nc.sync)
        load_xt(4, nc.scalar)
        load_bm_part(2, 0, 8, nc.sync)
        load_bm_part(2, 8, 16, nc.scalar)
        load_xt(5, nc.sync)
        load_xt(6, nc.scalar)
        load_bm_part(3, 0, 8, nc.sync)
        load_bm_part(3, 8, 16, nc.scalar)
        load_xt(7, nc.sync)

        def xt_slice(k, lo, hi):
            return xts[k][:, lo:hi]

        PH1 = 4  # m-tiles processed k-outer during the input-load ramp

        def emit_mm(ps, m, k, lo, hi):
            nc.tensor.matmul(
                ps[:],
                bms[m][:, k * 128 : (k + 1) * 128],
                xt_slice(k, lo, hi),
                start=(k == 0),
                stop=(k == KT - 1),
            )

        def emit_scan_out(m, psA, psB):
            ht = ht_pool.tile([128, T_SPAN], f32, tag="ht", name=f"ht{m}")
            a_bc = a_sb[:, m : m + 1].broadcast_to([128, HALF])
            nc.vector.tensor_tensor_scan(
                ht[:, 0:HALF], a_bc, psA[:], 0.0,
                mybir.AluOpType.mult, mybir.AluOpType.add,
            )
            rings[m % 2].dma_start(out=out[m, :, 0 : HALF - W], in_=ht[:, W:HALF])
            if m < MT - 1:
                nc.vector.tensor_tensor_scan(
                    ht[:, HALF:T_SPAN], a_bc, psB[:], ht[:, HALF - 1 : HALF],
                    mybir.AluOpType.mult, mybir.AluOpType.add,
                )
                rings[m % 2].dma_start(
                    out=out[m, :, HALF - W : T_CHUNK], in_=ht[:, HALF:T_SPAN]
                )
            else:
                # last m-tile: split the trailing scan+store so the final
                # dependency chain after the last matmul is half as long
                q3 = HALF + HALF // 2
                a_bc_h = a_sb[:, m : m + 1].broadcast_to([128, HALF // 2])
                nc.vector.tensor_tensor_scan(
                    ht[:, HALF:q3], a_bc_h, psB[:, 0 : HALF // 2],
                    ht[:, HALF - 1 : HALF],
                    mybir.AluOpType.mult, mybir.AluOpType.add,
                )
                rings[m % 2].dma_start(
                    out=out[m, :, HALF - W : q3 - W], in_=ht[:, HALF:q3]
                )
                nc.vector.tensor_tensor_scan(
                    ht[:, q3:T_SPAN], a_bc_h, psB[:, HALF // 2 : HALF],
                    ht[:, q3 - 1 : q3],
                    mybir.AluOpType.mult, mybir.AluOpType.add,
                )
                rings[(m + 1) % 2].dma_start(
                    out=out[m, :, q3 - W : T_CHUNK], in_=ht[:, q3:T_SPAN]
                )

        # Phase 1: m-tiles 0..3 accumulate k-outer following the xT piece
        # arrival order; m joins the rotation one piece late per index
        # (its B slab lands that much later) and catches up on the
        # already-resident backlog pieces.
        # HAM warm-up filler matmuls on bf16 zeros; interleaved into the
        # first phase-1 pieces they bridge the DMA-paced gaps so the PE
        # clock gate reaches 2.4 GHz early.
        warm = const.tile([128, HALF], mybir.dt.bfloat16)
        nc.gpsimd.memset(warm, 0.0)
        ps_warm = ps_gemm.tile([128, HALF], f32, tag="ps", name="ps_warm")
        ph1 = {}
        for m in range(PH1):
            ph1[m] = (
                ps_gemm.tile([128, HALF], f32, tag="ps", name=f"psA{m}"),
                ps_gemm.tile([128, HALF], f32, tag="ps", name=f"psB{m}"),
            )
        FILLERS = {0: 3, 1: 3, 2: 2, 3: 2, 4: 1, 5: 1}
        emitted = [0] * PH1  # next k to emit per phase-1 m
        for q in range(XP):
            avail_k = q + 1
            for m in range(min(q // 2 + 1, PH1)):
                for k in range(emitted[m], avail_k):
                    emit_mm(ph1[m][0], m, k, 0, HALF)
                    emit_mm(ph1[m][1], m, k, HALF, T_SPAN)
                emitted[m] = avail_k
            for _ in range(FILLERS.get(q, 0)):
                nc.tensor.matmul(
                    ps_warm[:], warm[:, 0:128], warm[:], start=True, stop=True
                )
        # prefetch the first phase-2 B slabs while phase-1 finishes
        load_bm_part(PH1, 0, 8, nc.sync)
        load_bm_part(PH1, 8, 16, nc.scalar)
        load_bm_part(PH1 + 1, 0, 8, nc.sync)
        load_bm_part(PH1 + 1, 8, 16, nc.scalar)
        for m in range(PH1):
            emit_scan_out(m, *ph1[m])

        # Phase 2: remaining m-tiles run dense, k-inner; B slabs stream
        # two m ahead, alternating rings.
        load_bm_part(PH1 + 2, 0, 8, nc.sync)
        load_bm_part(PH1 + 2, 8, 16, nc.scalar)
        for m in range(PH1, MT):
            if m + 3 < MT:
                load_bm_part(m + 3, 0, 8, rings[m % 2])
                load_bm_part(m + 3, 8, 16, rings[(m + 1) % 2])
            psA = ps_gemm.tile([128, HALF], f32, tag="ps", name=f"psA{m}")
            psB = ps_gemm.tile([128, HALF], f32, tag="ps", name=f"psB{m}")
            for k in range(KT):
                emit_mm(psA, m, k, 0, HALF)
            for k in range(KT):
                emit_mm(psB, m, k, HALF, T_SPAN)
            emit_scan_out(m, psA, psB)

    nc.compile()
    return nc


def _get_nc():
    if "nc" not in _CACHE:
        _CACHE["nc"] = _build()
    return _CACHE["nc"]


def _shard_inputs(x, a, B):
    x = np.ascontiguousarray(x, dtype=np.float32)
    a = np.ascontiguousarray(a, dtype=np.float32)
    B = np.ascontiguousarray(B, dtype=np.float32)
    B_lin = np.ascontiguousarray(
        B.reshape(KT, 128, MT, 128).transpose(2, 1, 0, 3)
    )  # [m, p, k, c] = B[128k+p, 128m+c]
    a_lin = np.ascontiguousarray(a.reshape(MT, 128).T)  # [p, m] = a[128m+p]
    xp = np.concatenate([np.zeros((W, H), np.float32), x], axis=0)
    in_maps = []
    for c in range(N_CORES):
        chunk = xp[c * T_CHUNK : c * T_CHUNK + T_SPAN]  # (T_SPAN, H)
        xT_lin = np.ascontiguousarray(
            chunk.T.reshape(KT, 128, T_SPAN).transpose(1, 0, 2)
        )  # [p, k, t] = x[t, 128k+p]
        in_maps.append({"xT": xT_lin, "Bm": B_lin, "a": a_lin})
    return in_maps


def _gather_output(results):
    out = np.empty((T_FULL, H), np.float32)
    for c in range(N_CORES):
        o = results[c]["out"]  # (MT, 128, T_CHUNK): h^T[chan, t_local]
        out[c * T_CHUNK : (c + 1) * T_CHUNK] = o.reshape(H, T_CHUNK).T
    return out[None]


def _run(inputs, trace=False):
    from concourse import bass_utils

    nc = _get_nc()
    in_maps = _shard_inputs(inputs["x"], inputs["a"], inputs["B"])
    res = bass_utils.run_bass_kernel_spmd(
        nc, in_maps, core_ids=list(range(N_CORES)), trace=trace
    )
    return _gather_output(res.results), res


def kernel(x, a, B):
    out, _ = _run({"x": x, "a": a, "B": B})
    return out



# revision 2
# speedup vs baseline: 1.1005x; 1.1005x over previous
"""Trainium2 Bass kernel for nn_DiagSSMBlock.

Math: s = x @ B  (T=4096, H=2048); h_t = a * h_{t-1} + s_t per channel
(equivalent to the reference depthwise causal conv with kernel a^t, since
|a| <= sqrt(2/H) ~= 0.031 the kernel decays below fp32 denormals within
~16 taps).  Output: (1, T, H).

Sharding: data-parallel over T across 8 cores; each core computes 512
timesteps (plus W=8 warm-up rows to rebuild the scan carry; a^9 ~ 3e-14
makes the truncation error ~1e-13, far under the 2e-2 gate).  Every core
streams the full B.

v2 changes vs the fp32r baseline:
  - x and B are converted to bf16 on the host.  The PE streams 1
    column/cycle for both fp32r and bf16, so GEMM time is unchanged, but
    DMA bytes halve (B: 16.8 -> 8.4 MB per core) and bf16 enables the
    compiler's fast-weight-load path (LDWEIGHTS ~53ns vs 107ns fp32r,
    now fully hidden under the ~110ns matmuls).
  - An up-front burst of filler matmuls (on a memset tile, no DMA deps)
    warms the PE HAM clock gate during the initial DMA ramp; the fp32r
    baseline ran its first ~12us of real matmuls at 1.2 GHz.
  - W: 16 -> 8 warm-up rows.

Per-core device pipeline:
  - x chunk is pre-transposed on the host (sharding layout prep) into
    xT[p, k, t] = x[t, 128k + p], so the GEMM contraction dim lands on
    SBUF partitions with no on-device transpose.
  - GEMM: for each of 16 output-channel tiles m, accumulate 16 k-tile
    matmuls into PSUM (bf16 operands, fp32 accumulate, moving free dim
    260 >= 256 -> full PE rate).
  - Scan: tensor_tensor_scan (DVE) state = a*state + s straight out of
    PSUM into SBUF, chained across the two 260-wide chunks.
  - Output stays channel-major (h^T) on device; the host unshard
    restores (T, H) layout while gathering the 8 T-chunks.
"""

from contextlib import ExitStack

import numpy as np

T_FULL, H = 4096, 2048
N_CORES = 8
T_CHUNK = T_FULL // N_CORES  # 512
W = 8  # scan warm-up rows
T_SPAN = T_CHUNK + W  # 520
HALF = T_SPAN // 2  # 260 (>= 256 keeps matmul at full rate)
KT = H // 128  # 16 contraction tiles
MT = H // 128  # 16 output-channel tiles
XP = 16  # xT arrives as 16 single-slab pieces (fine-grained sems)
N_WARM = 12  # up-front HAM warm-up filler matmuls

_CACHE = {}


def _build():
    import concourse.mybir as mybir
    import concourse.tile as tile
    from concourse import bacc

    f32 = mybir.dt.float32
    bf16 = mybir.dt.bfloat16

    nc = bacc.Bacc("TRN2", target_bir_lowering=False, debug=False, num_devices=N_CORES)
    xT = nc.dram_tensor("xT", [128, KT, T_SPAN], bf16, kind="ExternalInput").ap()
    Bm = nc.dram_tensor("Bm", [MT, 128, KT, 128], bf16, kind="ExternalInput").ap()
    a = nc.dram_tensor("a", [128, MT], f32, kind="ExternalInput").ap()
    out = nc.dram_tensor("out", [MT, 128, T_CHUNK], f32, kind="ExternalOutput").ap()

    with tile.TileContext(nc) as tc, ExitStack() as ctx:
        const = ctx.enter_context(tc.tile_pool(name="const", bufs=1))
        xt_pool = ctx.enter_context(tc.tile_pool(name="xt", bufs=XP))
        b_pool = ctx.enter_context(tc.tile_pool(name="bm", bufs=8))
        ht_pool = ctx.enter_context(tc.tile_pool(name="ht", bufs=3))
        ps_gemm = ctx.enter_context(tc.tile_pool(name="psg", bufs=8, space="PSUM"))

        a_sb = const.tile([128, MT], f32)
        nc.sync.dma_start(out=a_sb, in_=a)

        rings = [nc.sync, nc.scalar]

        bms = {}
        xts = [None] * XP

        def load_xt(q2, ring):
            # two single-slab DMAs in the same FIFO slot the former
            # 2-slab piece occupied: byte-identical ring flow, but the
            # first slab's consumers unblock one slab earlier
            for k in (2 * q2, 2 * q2 + 1):
                t = xt_pool.tile([128, T_SPAN], bf16, tag="xt", name=f"xt{k}")
                ring.dma_start(out=t[:], in_=xT[:, k, :])
                xts[k] = t

        def load_bm_part(m, lo, hi, ring):
            if m not in bms:
                bms[m] = b_pool.tile([128, KT * 128], bf16, tag="bm", name=f"bm{m}")
            ring.dma_start(
                out=bms[m][:, lo * 128 : hi * 128].rearrange(
                    "p (k c) -> p k c", k=hi - lo
                ),
                in_=Bm[m, :, lo:hi, :],
            )

        # Ring FIFO plan: first matmul needs bm0[k0:2] + xt piece 0; B
        # slabs for m=1..3 drip in between xT pieces so each m-tile can
        # join the phase-1 interleave shortly after the previous.
        load_bm_part(0, 0, 2, nc.sync)
        load_xt(0, nc.scalar)
        load_bm_part(0, 2, 8, nc.sync)
        load_bm_part(0, 8, 16, nc.scalar)
        load_xt(1, nc.sync)
        load_xt(2, nc.scalar)
        load_bm_part(1, 0, 8, nc.sync)
        load_bm_part(1, 8, 16, nc.scalar)
        load_xt(3, nc.sync)
        load_xt(4, nc.scalar)
        load_bm_part(2, 0, 8, nc.sync)
        load_bm_part(2, 8, 16, nc.scalar)
        load_xt(5, nc.sync)
        load_xt(6, nc.scalar)
        load_bm_part(3, 0, 8, nc.sync)
        load_bm_part(3, 8, 16, nc.scalar)
        load_xt(7, nc.sync)

        def xt_slice(k, lo, hi):
            return xts[k][:, lo:hi]

        PH1 = 4  # m-tiles processed k-outer during the input-load ramp

        def emit_mm(ps, m, k, lo, hi):
            nc.tensor.matmul(
                ps[:],
                bms[m][:, k * 128 : (k + 1) * 128],
                xt_slice(k, lo, hi),
                start=(k == 0),
                stop=(k == KT - 1),
            )

        def emit_scan_out(m, psA, psB):
            ht = ht_pool.tile([128, T_SPAN], f32, tag="ht", name=f"ht{m}")
            a_bc = a_sb[:, m : m + 1].broadcast_to([128, HALF])
            nc.vector.tensor_tensor_scan(
                ht[:, 0:HALF], a_bc, psA[:], 0.0,
                mybir.AluOpType.mult, mybir.AluOpType.add,
            )
            rings[m % 2].dma_start(out=out[m, :, 0 : HALF - W], in_=ht[:, W:HALF])
            if m < MT - 1:
                nc.vector.tensor_tensor_scan(
                    ht[:, HALF:T_SPAN], a_bc, psB[:], ht[:, HALF - 1 : HALF],
                    mybir.AluOpType.mult, mybir.AluOpType.add,
                )
                rings[m % 2].dma_start(
                    out=out[m, :, HALF - W : T_CHUNK], in_=ht[:, HALF:T_SPAN]
                )
            else:
                # last m-tile: split the trailing scan+store so the final
                # dependency chain after the last matmul is half as long
                q3 = HALF + HALF // 2
                a_bc_h = a_sb[:, m : m + 1].broadcast_to([128, HALF // 2])
                nc.vector.tensor_tensor_scan(
                    ht[:, HALF:q3], a_bc_h, psB[:, 0 : HALF // 2],
                    ht[:, HALF - 1 : HALF],
                    mybir.AluOpType.mult, mybir.AluOpType.add,
                )
                rings[m % 2].dma_start(
                    out=out[m, :, HALF - W : q3 - W], in_=ht[:, HALF:q3]
                )
                nc.vector.tensor_tensor_scan(
                    ht[:, q3:T_SPAN], a_bc_h, psB[:, HALF // 2 : HALF],
                    ht[:, q3 - 1 : q3],
                    mybir.AluOpType.mult, mybir.AluOpType.add,
                )
                rings[(m + 1) % 2].dma_start(
                    out=out[m, :, q3 - W : T_CHUNK], in_=ht[:, q3:T_SPAN]
                )

        # Up-front HAM warm-up: filler matmuls on a memset bf16 tile with
        # no DMA dependency.  They run while the input DMAs ramp, so the
        # clock gate reaches 2.4 GHz before the first real matmul.
        warm = const.tile([128, 512], bf16)
        nc.gpsimd.memset(warm, 0.0)
        ps_warm = ps_gemm.tile([128, 512], f32, tag="ps", name="ps_warm")
        for _ in range(N_WARM):
            nc.tensor.matmul(
                ps_warm[:], warm[:, 0:128], warm[:], start=True, stop=True
            )

        # Phase 1: m-tiles 0..3 accumulate k-outer following the xT piece
        # arrival order; m joins the rotation one piece late per index
        # (its B slab lands that much later) and catches up on the
        # already-resident backlog pieces.
        ph1 = {}
        for m in range(PH1):
            ph1[m] = (
                ps_gemm.tile([128, HALF], f32, tag="ps", name=f"psA{m}"),
                ps_gemm.tile([128, HALF], f32, tag="ps", name=f"psB{m}"),
            )
        emitted = [0] * PH1  # next k to emit per phase-1 m
        for q in range(XP):
            avail_k = q + 1
            for m in range(min(q // 2 + 1, PH1)):
                for k in range(emitted[m], avail_k):
                    emit_mm(ph1[m][0], m, k, 0, HALF)
                    emit_mm(ph1[m][1], m, k, HALF, T_SPAN)
                emitted[m] = avail_k
        # prefetch the first phase-2 B slabs while phase-1 finishes
        load_bm_part(PH1, 0, 8, nc.sync)
        load_bm_part(PH1, 8, 16, nc.scalar)
        load_bm_part(PH1 + 1, 0, 8, nc.sync)
        load_bm_part(PH1 + 1, 8, 16, nc.scalar)
        for m in range(PH1):
            emit_scan_out(m, *ph1[m])

        # Phase 2: remaining m-tiles run dense, k-inner; B slabs stream
        # two m ahead, alternating rings.
        load_bm_part(PH1 + 2, 0, 8, nc.sync)
        load_bm_part(PH1 + 2, 8, 16, nc.scalar)
        for m in range(PH1, MT):
            if m + 3 < MT:
                load_bm_part(m + 3, 0, 8, rings[m % 2])
                load_bm_part(m + 3, 8, 16, rings[(m + 1) % 2])
            psA = ps_gemm.tile([128, HALF], f32, tag="ps", name=f"psA{m}")
            psB = ps_gemm.tile([128, HALF], f32, tag="ps", name=f"psB{m}")
            for k in range(KT):
                emit_mm(psA, m, k, 0, HALF)
            for k in range(KT):
                emit_mm(psB, m, k, HALF, T_SPAN)
            emit_scan_out(m, psA, psB)

    nc.compile()
    return nc


def _get_nc():
    if "nc" not in _CACHE:
        _CACHE["nc"] = _build()
    return _CACHE["nc"]


def _shard_inputs(x, a, B):
    import ml_dtypes

    bf16 = ml_dtypes.bfloat16
    x = np.ascontiguousarray(x, dtype=np.float32)
    a = np.ascontiguousarray(a, dtype=np.float32)
    B = np.ascontiguousarray(B, dtype=np.float32)
    B_lin = np.ascontiguousarray(
        B.reshape(KT, 128, MT, 128).transpose(2, 1, 0, 3).astype(bf16)
    )  # [m, p, k, c] = B[128k+p, 128m+c]
    a_lin = np.ascontiguousarray(a.reshape(MT, 128).T)  # [p, m] = a[128m+p]
    xp = np.concatenate([np.zeros((W, H), np.float32), x], axis=0).astype(bf16)
    in_maps = []
    for c in range(N_CORES):
        chunk = xp[c * T_CHUNK : c * T_CHUNK + T_SPAN]  # (T_SPAN, H)
        xT_lin = np.ascontiguousarray(
            chunk.T.reshape(KT, 128, T_SPAN).transpose(1, 0, 2)
        )  # [p, k, t] = x[t, 128k+p]
        in_maps.append({"xT": xT_lin, "Bm": B_lin, "a": a_lin})
    return in_maps


def _gather_output(results):
    out = np.empty((T_FULL, H), np.float32)
    for c in range(N_CORES):
        o = results[c]["out"]  # (MT, 128, T_CHUNK): h^T[chan, t_local]
        out[c * T_CHUNK : (c + 1) * T_CHUNK] = o.reshape(H, T_CHUNK).T
    return out[None]


def _run(inputs, trace=False):
    from concourse import bass_utils

    nc = _get_nc()
    in_maps = _shard_inputs(inputs["x"], inputs["a"], inputs["B"])
    res = bass_utils.run_bass_kernel_spmd(
        nc, in_maps, core_ids=list(range(N_CORES)), trace=trace
    )
    return _gather_output(res.results), res


def kernel(x, a, B):
    out, _ = _run({"x": x, "a": a, "B": B})
    return out


# revision 3
# speedup vs baseline: 1.1360x; 1.0323x over previous
"""Trainium2 Bass kernel for nn_DiagSSMBlock.

Math: s = x @ B  (T=4096, H=2048); h_t = a * h_{t-1} + s_t per channel
(equivalent to the reference depthwise causal conv with kernel a^t, since
|a| <= sqrt(2/H) ~= 0.031 the kernel decays below fp32 denormals within
~16 taps).  Output: (1, T, H).

Sharding: data-parallel over T across 8 cores; each core computes 512
timesteps (plus W=8 warm-up rows to rebuild the scan carry; a^9 ~ 3e-14
makes the truncation error ~1e-13, far under the 2e-2 gate).  Every core
streams the full B.

v3 design notes (measured on HW):
  - x/B in bf16: PE streams 1 column/cycle for fp32r and bf16 alike, so
    GEMM time is unchanged, but DMA bytes halve and LDWEIGHTS uses the
    fast-weight-load path (~53ns, fully hidden under ~110ns matmuls).
  - Each dma_start costs ~0.7us of descriptor-gen on the issuing HWDGE
    engine (sync/scalar), so DMAs are few and large: 4 quad-slab xT
    pieces, 1 DMA per B m-tile, 1 output DMA per m-tile.
  - Up-front filler-matmul burst (memset tile, no DMA deps) warms the
    PE HAM clock gate during the ~7us framework preamble + DMA ramp.
  - Output is written bf16 (scan output dtype) and widened to fp32 on
    the host: halves output DMA bytes; rounding error ~0.4% of |h|,
    well under the 2e-2 gate.

Per-core device pipeline:
  - x chunk is pre-transposed on the host (sharding layout prep) into
    xT[p, k, t] = x[t, 128k + p], so the GEMM contraction dim lands on
    SBUF partitions with no on-device transpose.
  - GEMM: for each of 16 output-channel tiles m, accumulate 16 k-tile
    matmuls into PSUM (bf16 operands, fp32 accumulate, moving free dim
    260 >= 256 -> full PE rate).
  - Scan: tensor_tensor_scan (DVE) state = a*state + s straight out of
    PSUM into SBUF, chained across the two 260-wide chunks.
  - Output stays channel-major (h^T) on device; the host unshard
    restores (T, H) layout while gathering the 8 T-chunks.
"""

from contextlib import ExitStack

import numpy as np

T_FULL, H = 4096, 2048
N_CORES = 8
T_CHUNK = T_FULL // N_CORES  # 512
W = 8  # scan warm-up rows
T_SPAN = T_CHUNK + W  # 520
HALF = T_SPAN // 2  # 260 (>= 256 keeps matmul at full rate)
KT = H // 128  # 16 contraction tiles
MT = H // 128  # 16 output-channel tiles
QP = 4  # xT arrives as 4 quad-slab pieces
PH1 = 4  # m-tiles processed k-outer during the input-load ramp
N_WARM = 12  # up-front HAM warm-up filler matmuls

_CACHE = {}


def _build():
    import concourse.mybir as mybir
    import concourse.tile as tile
    from concourse import bacc

    f32 = mybir.dt.float32
    bf16 = mybir.dt.bfloat16

    nc = bacc.Bacc("TRN2", target_bir_lowering=False, debug=False, num_devices=N_CORES)
    xT = nc.dram_tensor("xT", [128, KT, T_SPAN], bf16, kind="ExternalInput").ap()
    Bm = nc.dram_tensor("Bm", [MT, 128, KT, 128], bf16, kind="ExternalInput").ap()
    a = nc.dram_tensor("a", [128, MT], f32, kind="ExternalInput").ap()
    out = nc.dram_tensor("out", [MT, 128, T_CHUNK], bf16, kind="ExternalOutput").ap()

    with tile.TileContext(nc) as tc, ExitStack() as ctx:
        const = ctx.enter_context(tc.tile_pool(name="const", bufs=1))
        xt_pool = ctx.enter_context(tc.tile_pool(name="xt", bufs=QP))
        b_pool = ctx.enter_context(tc.tile_pool(name="bm", bufs=MT))
        ht_pool = ctx.enter_context(tc.tile_pool(name="ht", bufs=4))
        ps_gemm = ctx.enter_context(tc.tile_pool(name="psg", bufs=8, space="PSUM"))

        rings = [nc.sync, nc.scalar]

        bms = {}
        xqs = [None] * QP

        def load_xt(q, ring):
            t = xt_pool.tile([128, 4 * T_SPAN], bf16, tag="xt", name=f"xq{q}")
            ring.dma_start(
                out=t[:].rearrange("p (k t) -> p k t", k=4),
                in_=xT[:, 4 * q : 4 * q + 4, :],
            )
            xqs[q] = t

        def load_bm(m, ring):
            bms[m] = b_pool.tile([128, KT * 128], bf16, tag="bm", name=f"bm{m}")
            ring.dma_start(
                out=bms[m][:].rearrange("p (k c) -> p k c", k=KT),
                in_=Bm[m, :, :, :],
            )

        # Ring plan: sync carries B m-tiles, scalar carries a + the four
        # xT quad-pieces; both rings then alternate the output stores.
        a_sb = const.tile([128, MT], f32)
        nc.scalar.dma_start(out=a_sb, in_=a)
        load_bm(0, nc.sync)
        load_xt(0, nc.scalar)
        load_bm(1, nc.sync)
        load_xt(1, nc.scalar)
        load_bm(2, nc.sync)
        load_xt(2, nc.scalar)
        load_bm(3, nc.sync)
        load_xt(3, nc.scalar)
        load_bm(4, nc.sync)
        load_bm(5, nc.scalar)
        load_bm(6, nc.sync)

        def xt_slice(k, lo, hi):
            return xqs[k // 4][:, (k % 4) * T_SPAN + lo : (k % 4) * T_SPAN + hi]

        def emit_mm(ps, m, k, lo, hi):
            nc.tensor.matmul(
                ps[:],
                bms[m][:, k * 128 : (k + 1) * 128],
                xt_slice(k, lo, hi),
                start=(k == 0),
                stop=(k == KT - 1),
            )

        def emit_scan_out(m, psA, psB):
            ht = ht_pool.tile([128, T_SPAN], bf16, tag="ht", name=f"ht{m}")
            a_bc = a_sb[:, m : m + 1].broadcast_to([128, HALF])
            nc.vector.tensor_tensor_scan(
                ht[:, 0:HALF], a_bc, psA[:], 0.0,
                mybir.AluOpType.mult, mybir.AluOpType.add,
            )
            if m < MT - 1:
                nc.vector.tensor_tensor_scan(
                    ht[:, HALF:T_SPAN], a_bc, psB[:], ht[:, HALF - 1 : HALF],
                    mybir.AluOpType.mult, mybir.AluOpType.add,
                )
                rings[m % 2].dma_start(
                    out=out[m, :, :], in_=ht[:, W:T_SPAN]
                )
            else:
                # last m-tile: store the first half as soon as its scan is
                # done and split the trailing scan+store so the final
                # dependency chain after the last matmul is short
                rings[m % 2].dma_start(
                    out=out[m, :, 0 : HALF - W], in_=ht[:, W:HALF]
                )
                q3 = HALF + HALF // 2
                a_bc_h = a_sb[:, m : m + 1].broadcast_to([128, HALF // 2])
                nc.vector.tensor_tensor_scan(
                    ht[:, HALF:q3], a_bc_h, psB[:, 0 : HALF // 2],
                    ht[:, HALF - 1 : HALF],
                    mybir.AluOpType.mult, mybir.AluOpType.add,
                )
                rings[m % 2].dma_start(
                    out=out[m, :, HALF - W : q3 - W], in_=ht[:, HALF:q3]
                )
                nc.vector.tensor_tensor_scan(
                    ht[:, q3:T_SPAN], a_bc_h, psB[:, HALF // 2 : HALF],
                    ht[:, q3 - 1 : q3],
                    mybir.AluOpType.mult, mybir.AluOpType.add,
                )
                rings[(m + 1) % 2].dma_start(
                    out=out[m, :, q3 - W : T_CHUNK], in_=ht[:, q3:T_SPAN]
                )

        # Up-front HAM warm-up: filler matmuls on a memset bf16 tile with
        # no DMA dependency.  They run while the input DMAs ramp, so the
        # clock gate reaches 2.4 GHz before the first real matmul.
        warm = const.tile([128, 512], bf16)
        nc.gpsimd.memset(warm, 0.0)
        ps_warm = ps_gemm.tile([128, 512], f32, tag="ps", name="ps_warm")
        for _ in range(N_WARM):
            nc.tensor.matmul(
                ps_warm[:], warm[:, 0:128], warm[:], start=True, stop=True
            )

        # Phase 1: m-tiles 0..3 accumulate k-outer following the xT
        # quad-piece arrival order; m joins the rotation one piece late
        # per index (its B tile lands that much later) and catches up on
        # the already-resident backlog.
        ph1 = {}
        for m in range(PH1):
            ph1[m] = (
                ps_gemm.tile([128, HALF], f32, tag="ps", name=f"psA{m}"),
                ps_gemm.tile([128, HALF], f32, tag="ps", name=f"psB{m}"),
            )
        emitted = [0] * PH1  # next k to emit per phase-1 m
        for q in range(QP):
            avail_k = 4 * (q + 1)
            for m in range(min(q + 1, PH1)):
                for k in range(emitted[m], avail_k):
                    emit_mm(ph1[m][0], m, k, 0, HALF)
                    emit_mm(ph1[m][1], m, k, HALF, T_SPAN)
                emitted[m] = avail_k
        for m in range(PH1):
            emit_scan_out(m, *ph1[m])

        # Phase 2: remaining m-tiles run dense, k-inner; B tiles stream
        # three m ahead, alternating rings.
        for m in range(PH1, MT):
            if m + 3 < MT and m + 3 > 6:
                load_bm(m + 3, rings[m % 2])
            psA = ps_gemm.tile([128, HALF], f32, tag="ps", name=f"psA{m}")
            psB = ps_gemm.tile([128, HALF], f32, tag="ps", name=f"psB{m}")
            for k in range(KT):
                emit_mm(psA, m, k, 0, HALF)
            for k in range(KT):
                emit_mm(psB, m, k, HALF, T_SPAN)
            emit_scan_out(m, psA, psB)

    nc.compile()
    return nc


def _get_nc():
    if "nc" not in _CACHE:
        _CACHE["nc"] = _build()
    return _CACHE["nc"]


def _shard_inputs(x, a, B):
    import ml_dtypes

    bf16 = ml_dtypes.bfloat16
    x = np.ascontiguousarray(x, dtype=np.float32)
    a = np.ascontiguousarray(a, dtype=np.float32)
    B = np.ascontiguousarray(B, dtype=np.float32)
    B_lin = np.ascontiguousarray(
        B.reshape(KT, 128, MT, 128).transpose(2, 1, 0, 3).astype(bf16)
    )  # [m, p, k, c] = B[128k+p, 128m+c]
    a_lin = np.ascontiguousarray(a.reshape(MT, 128).T)  # [p, m] = a[128m+p]
    xp = np.concatenate([np.zeros((W, H), np.float32), x], axis=0).astype(bf16)
    in_maps = []
    for c in range(N_CORES):
        chunk = xp[c * T_CHUNK : c * T_CHUNK + T_SPAN]  # (T_SPAN, H)
        xT_lin = np.ascontiguousarray(
            chunk.T.reshape(KT, 128, T_SPAN).transpose(1, 0, 2)
        )  # [p, k, t] = x[t, 128k+p]
        in_maps.append({"xT": xT_lin, "Bm": B_lin, "a": a_lin})
    return in_maps


def _gather_output(results):
    out = np.empty((T_FULL, H), np.float32)
    for c in range(N_CORES):
        o = np.asarray(results[c]["out"], dtype=np.float32)  # (MT, 128, T_CHUNK)
        out[c * T_CHUNK : (c + 1) * T_CHUNK] = o.reshape(H, T_CHUNK).T
    return out[None]


def _run(inputs, trace=False):
    from concourse import bass_utils

    nc = _get_nc()
    in_maps = _shard_inputs(inputs["x"], inputs["a"], inputs["B"])
    res = bass_utils.run_bass_kernel_spmd(
        nc, in_maps, core_ids=list(range(N_CORES)), trace=trace
    )
    return _gather_output(res.results), res


def kernel(x, a, B):
    out, _ = _run({"x": x, "a": a, "B": B})
    return out


# revision 5
# speedup vs baseline: 1.1526x; 1.0147x over previous
"""Trainium2 Bass kernel for nn_DiagSSMBlock.

Math: s = x @ B  (T=4096, H=2048); h_t = a * h_{t-1} + s_t per channel
(equivalent to the reference depthwise causal conv with kernel a^t, since
|a| <= sqrt(2/H) ~= 0.031 the kernel decays below fp32 denormals within
~16 taps).  Output: (1, T, H).

Sharding: data-parallel over T across 8 cores; each core computes 512
timesteps (plus W=4 warm-up rows to rebuild the scan carry; a^5 ~ 3e-8
makes the truncation error ~1e-7, far under the 2e-2 gate).  Every core
streams the full B.

Measured-design notes (HW traces):
  - x/B in bf16: PE streams 1 column/cycle for fp32r and bf16 alike, so
    GEMM time is unchanged, but DMA bytes halve and LDWEIGHTS uses the
    fast-weight-load path (~97ns, fully hidden under ~111ns matmuls).
  - Each dma_start costs ~0.65us of descriptor-gen on the issuing HWDGE
    engine (sync/scalar) and the two rings share ~300-358 GB/s, so the
    load plan interleaves both rings in PE-consumption order: B tiles
    for m0/m1 split in half, x arriving as eight 2-slab pieces.
  - Up-front filler-matmul burst (memset tile, no DMA deps) warms the
    PE HAM clock gate during the ~7us framework preamble + DMA ramp.
  - Output is written bf16 (scan output dtype) and widened to fp32 on
    the host: halves output DMA bytes; rounding error ~0.4% of |h|.
  - DVE ops have ~390ns fixed overhead -> one scan per PSUM half, one
    output DMA per m-tile (the last tile is split for tail latency).

Per-core device pipeline:
  - x chunk is pre-transposed on the host (sharding layout prep) into
    xT[p, k, t] = x[t, 128k + p], so the GEMM contraction dim lands on
    SBUF partitions with no on-device transpose.
  - GEMM: for each of 16 output-channel tiles m, accumulate 16 k-tile
    matmuls into PSUM (bf16 operands, fp32 accumulate, moving free dim
    258 >= 256 -> full PE rate).
  - Scan: tensor_tensor_scan (DVE) state = a*state + s straight out of
    PSUM into SBUF, chained across the two 258-wide chunks.
  - Output stays channel-major (h^T) on device; the host unshard
    restores (T, H) layout while gathering the 8 T-chunks.
"""

from contextlib import ExitStack

import numpy as np

T_FULL, H = 4096, 2048
N_CORES = 8
T_CHUNK = T_FULL // N_CORES  # 512
W = 4  # scan warm-up rows
T_SPAN = T_CHUNK + W  # 516
HALF = T_SPAN // 2  # 258 (>= 256 keeps matmul at full rate)
KT = H // 128  # 16 contraction tiles
MT = H // 128  # 16 output-channel tiles
NP = 8  # xT arrives as 8 two-slab pieces
N_WARM = 8  # up-front HAM warm-up filler matmuls (N=512 each)

_CACHE = {}


def _build():
    import concourse.mybir as mybir
    import concourse.tile as tile
    from concourse import bacc

    f32 = mybir.dt.float32
    bf16 = mybir.dt.bfloat16

    nc = bacc.Bacc("TRN2", target_bir_lowering=False, debug=False, num_devices=N_CORES)
    xT = nc.dram_tensor("xT", [128, KT, T_SPAN], bf16, kind="ExternalInput").ap()
    Bm = nc.dram_tensor("Bm", [MT, 128, KT, 128], bf16, kind="ExternalInput").ap()
    a = nc.dram_tensor("a", [128, MT], f32, kind="ExternalInput").ap()
    out = nc.dram_tensor("out", [MT, 128, T_CHUNK], bf16, kind="ExternalOutput").ap()

    with tile.TileContext(nc) as tc, ExitStack() as ctx:
        const = ctx.enter_context(tc.tile_pool(name="const", bufs=1))
        xt_pool = ctx.enter_context(tc.tile_pool(name="xt", bufs=NP))
        b_pool = ctx.enter_context(tc.tile_pool(name="bm", bufs=MT))
        ht_pool = ctx.enter_context(tc.tile_pool(name="ht", bufs=4))
        ps_gemm = ctx.enter_context(tc.tile_pool(name="psg", bufs=8, space="PSUM"))

        rings = [nc.sync, nc.scalar]

        bms = {}
        xps = [None] * NP

        def load_xp(p, ring):
            t = xt_pool.tile([128, 2 * T_SPAN], bf16, tag="xt", name=f"xp{p}")
            ring.dma_start(
                out=t[:].rearrange("p (k t) -> p k t", k=2),
                in_=xT[:, 2 * p : 2 * p + 2, :],
            )
            xps[p] = t

        def load_bm(m, ring, lo=0, hi=KT):
            if m not in bms:
                bms[m] = b_pool.tile([128, KT * 128], bf16, tag="bm", name=f"bm{m}")
            ring.dma_start(
                out=bms[m][:, lo * 128 : hi * 128].rearrange(
                    "p (k c) -> p k c", k=hi - lo
                ),
                in_=Bm[m, :, lo:hi, :],
            )

        # Interleaved ring plan, ordered to match PE consumption: both
        # rings alternate x pieces and B tiles so neither the k-stream
        # nor the m-join order starves.
        a_sb = const.tile([128, MT], f32)
        load_bm(0, nc.sync, 0, 8)     # ring A
        nc.scalar.dma_start(out=a_sb, in_=a)  # ring B (tiny)
        load_xp(0, nc.scalar)
        load_xp(1, nc.sync)
        load_xp(2, nc.scalar)
        load_bm(1, nc.sync, 0, 8)
        load_bm(0, nc.scalar, 8, 16)
        load_xp(3, nc.sync)
        load_xp(4, nc.scalar)
        load_xp(5, nc.sync)
        load_bm(1, nc.scalar, 8, 16)
        load_xp(7, nc.sync)
        load_xp(6, nc.scalar)
        load_bm(2, nc.sync)
        load_bm(3, nc.scalar)
        load_bm(4, nc.sync)
        load_bm(5, nc.scalar)
        load_bm(6, nc.sync)

        def xt_slice(k, lo, hi):
            return xps[k // 2][:, (k % 2) * T_SPAN + lo : (k % 2) * T_SPAN + hi]

        def emit_mm(ps, m, k, lo, hi):
            nc.tensor.matmul(
                ps[:],
                bms[m][:, k * 128 : (k + 1) * 128],
                xt_slice(k, lo, hi),
                start=(k == 0),
                stop=(k == KT - 1),
            )

        def emit_scan_out(m, psA, psB):
            ht = ht_pool.tile([128, T_SPAN], bf16, tag="ht", name=f"ht{m}")
            a_bc = a_sb[:, m : m + 1].broadcast_to([128, HALF])
            nc.vector.tensor_tensor_scan(
                ht[:, 0:HALF], a_bc, psA[:], 0.0,
                mybir.AluOpType.mult, mybir.AluOpType.add,
            )
            if m < MT - 1:
                nc.vector.tensor_tensor_scan(
                    ht[:, HALF:T_SPAN], a_bc, psB[:], ht[:, HALF - 1 : HALF],
                    mybir.AluOpType.mult, mybir.AluOpType.add,
                )
                rings[m % 2].dma_start(
                    out=out[m, :, :], in_=ht[:, W:T_SPAN]
                )
            else:
                # last m-tile: store the first half as soon as its scan is
                # done and split the trailing scan+store so the final
                # dependency chain after the last matmul is short
                rings[m % 2].dma_start(
                    out=out[m, :, 0 : HALF - W], in_=ht[:, W:HALF]
                )
                q3 = HALF + HALF // 2
                a_bc_h = a_sb[:, m : m + 1].broadcast_to([128, HALF // 2])
                nc.vector.tensor_tensor_scan(
                    ht[:, HALF:q3], a_bc_h, psB[:, 0 : HALF // 2],
                    ht[:, HALF - 1 : HALF],
                    mybir.AluOpType.mult, mybir.AluOpType.add,
                )
                rings[m % 2].dma_start(
                    out=out[m, :, HALF - W : q3 - W], in_=ht[:, HALF:q3]
                )
                nc.vector.tensor_tensor_scan(
                    ht[:, q3:T_SPAN], a_bc_h, psB[:, HALF // 2 : HALF],
                    ht[:, q3 - 1 : q3],
                    mybir.AluOpType.mult, mybir.AluOpType.add,
                )
                rings[(m + 1) % 2].dma_start(
                    out=out[m, :, q3 - W : T_CHUNK], in_=ht[:, q3:T_SPAN]
                )

        # Up-front HAM warm-up: filler matmuls on a memset bf16 tile with
        # no DMA dependency.  They run while the input DMAs ramp, so the
        # clock gate reaches 2.4 GHz before the first real matmul.
        warm = const.tile([128, 512], bf16)
        nc.gpsimd.memset(warm, 0.0)
        ps_warm = ps_gemm.tile([128, 512], f32, tag="ps", name="ps_warm")
        for _ in range(N_WARM):
            nc.tensor.matmul(
                ps_warm[:], warm[:, 0:128], warm[:], start=True, stop=True
            )

        # Phase 1: m0/m1 follow the 2-slab x pieces (m1 one piece late),
        # m2 joins at piece 6, m3 catches up after the last piece.
        PH1 = 4
        JOIN = {0: 0, 1: 1, 2: 6, 3: 8}
        ph1 = {}
        for m in range(PH1):
            ph1[m] = (
                ps_gemm.tile([128, HALF], f32, tag="ps", name=f"psA{m}"),
                ps_gemm.tile([128, HALF], f32, tag="ps", name=f"psB{m}"),
            )
        emitted = [0] * PH1
        for p in range(NP):
            for m in range(PH1):
                if p >= JOIN[m]:
                    for k in range(emitted[m], 2 * (p + 1)):
                        emit_mm(ph1[m][0], m, k, 0, HALF)
                        emit_mm(ph1[m][1], m, k, HALF, T_SPAN)
                    emitted[m] = 2 * (p + 1)
        for m in range(PH1):
            for k in range(emitted[m], KT):
                emit_mm(ph1[m][0], m, k, 0, HALF)
                emit_mm(ph1[m][1], m, k, HALF, T_SPAN)
            emitted[m] = KT
        for m in range(PH1):
            emit_scan_out(m, *ph1[m])

        # Phase 2: remaining m-tiles run dense, k-inner; B tiles stream
        # three m ahead, alternating rings.
        for m in range(PH1, MT):
            if m + 3 < MT:
                load_bm(m + 3, rings[(m + 3) % 2])
            psA = ps_gemm.tile([128, HALF], f32, tag="ps", name=f"psA{m}")
            psB = ps_gemm.tile([128, HALF], f32, tag="ps", name=f"psB{m}")
            for k in range(KT):
                emit_mm(psA, m, k, 0, HALF)
            for k in range(KT):
                emit_mm(psB, m, k, HALF, T_SPAN)
            emit_scan_out(m, psA, psB)

    nc.compile()
    return nc


def _get_nc():
    if "nc" not in _CACHE:
        _CACHE["nc"] = _build()
    return _CACHE["nc"]


def _shard_inputs(x, a, B):
    import ml_dtypes

    bf16 = ml_dtypes.bfloat16
    x = np.ascontiguousarray(x, dtype=np.float32)
    a = np.ascontiguousarray(a, dtype=np.float32)
    B = np.ascontiguousarray(B, dtype=np.float32)
    B_lin = np.ascontiguousarray(
        B.reshape(KT, 128, MT, 128).transpose(2, 1, 0, 3).astype(bf16)
    )  # [m, p, k, c] = B[128k+p, 128m+c]
    a_lin = np.ascontiguousarray(a.reshape(MT, 128).T)  # [p, m] = a[128m+p]
    xp = np.concatenate([np.zeros((W, H), np.float32), x], axis=0).astype(bf16)
    in_maps = []
    for c in range(N_CORES):
        chunk = xp[c * T_CHUNK : c * T_CHUNK + T_SPAN]  # (T_SPAN, H)
        xT_lin = np.ascontiguousarray(
            chunk.T.reshape(KT, 128, T_SPAN).transpose(1, 0, 2)
        )  # [p, k, t] = x[t, 128k+p]
        in_maps.append({"xT": xT_lin, "Bm": B_lin, "a": a_lin})
    return in_maps


def _gather_output(results):
    out = np.empty((T_FULL, H), np.float32)
    for c in range(N_CORES):
        o = np.asarray(results[c]["out"], dtype=np.float32)  # (MT, 128, T_CHUNK)
        out[c * T_CHUNK : (c + 1) * T_CHUNK] = o.reshape(H, T_CHUNK).T
    return out[None]


def _run(inputs, trace=False):
    from concourse import bass_utils

    nc = _get_nc()
    in_maps = _shard_inputs(inputs["x"], inputs["a"], inputs["B"])
    res = bass_utils.run_bass_kernel_spmd(
        nc, in_maps, core_ids=list(range(N_CORES)), trace=trace
    )
    return _gather_output(res.results), res


def kernel(x, a, B):
    out, _ = _run({"x": x, "a": a, "B": B})
    return out


# revision 6
# speedup vs baseline: 1.1855x; 1.0285x over previous
"""Trainium2 Bass kernel for nn_DiagSSMBlock.

Math: s = x @ B  (T=4096, H=2048); h_t = a * h_{t-1} + s_t per channel
(equivalent to the reference depthwise causal conv with kernel a^t, since
|a| <= sqrt(2/H) ~= 0.031 the kernel decays below fp32 denormals within
~16 taps).  Output: (1, T, H).

Sharding: data-parallel over T across 8 cores; each core computes 512
timesteps (plus W=4 warm-up rows to rebuild the scan carry; a^5 ~ 3e-8
makes the truncation error ~1e-7, far under the 2e-2 gate).  Every core
streams the full B.

Measured-design notes (HW traces):
  - x/B in bf16: PE streams 1 column/cycle for fp32r and bf16 alike, so
    GEMM time is unchanged, but DMA bytes halve and LDWEIGHTS uses the
    fast-weight-load path (~97ns, fully hidden under ~111ns matmuls).
  - The early phase is supply-bound: the two HWDGE rings share the
    ~358 GB/s HBM-per-core cap, and each dma_start costs ~0.65us of
    descriptor-gen.  Phase 1 therefore runs THREE m-tiles over k-HALF
    blocks (PSUM accumulation groups stay open), so only xp0-3 +
    half-B-tiles gate the start; the supply plan interleaves both rings
    in exact PE-consumption order.
  - Up-front filler matmuls (memset tile -> psA0, reset by m0k0's
    start=True) warm the PE HAM clock gate during the ~7us framework
    preamble + DMA ramp.
  - Output is written bf16 and widened to fp32 on the host; rounding
    error ~0.4% of |h|, well under the 2e-2 gate.
  - DVE ops have ~390ns fixed overhead -> one scan per PSUM half, one
    output DMA per m-tile (the last tile is split for tail latency).

Per-core device pipeline:
  - x chunk is pre-transposed on the host (sharding layout prep) into
    xT[p, k, t] = x[t, 128k + p], so the GEMM contraction dim lands on
    SBUF partitions with no on-device transpose.
  - GEMM: for each of 16 output-channel tiles m, accumulate 16 k-tile
    matmuls into PSUM (bf16 operands, fp32 accumulate, moving free dim
    258 >= 256 -> full PE rate).
  - Scan: tensor_tensor_scan (DVE) state = a*state + s straight out of
    PSUM into SBUF, chained across the two 258-wide chunks.
  - Output stays channel-major (h^T) on device; the host unshard
    restores (T, H) layout while gathering the 8 T-chunks.
"""

from contextlib import ExitStack

import numpy as np

T_FULL, H = 4096, 2048
N_CORES = 8
T_CHUNK = T_FULL // N_CORES  # 512
W = 4  # scan warm-up rows
T_SPAN = T_CHUNK + W  # 516
HALF = T_SPAN // 2  # 258 (>= 256 keeps matmul at full rate)
KT = H // 128  # 16 contraction tiles
MT = H // 128  # 16 output-channel tiles
NP = 8  # xT arrives as 8 two-slab pieces
PH1 = 3  # phase-1 m-tiles (2 PSUM banks each, k-half blocks)
N_WARM = 14  # up-front HAM warm-up filler matmuls (N=258 each)

_CACHE = {}


def _build():
    import concourse.mybir as mybir
    import concourse.tile as tile
    from concourse import bacc

    f32 = mybir.dt.float32
    bf16 = mybir.dt.bfloat16

    nc = bacc.Bacc("TRN2", target_bir_lowering=False, debug=False, num_devices=N_CORES)
    xT = nc.dram_tensor("xT", [128, KT, T_SPAN], bf16, kind="ExternalInput").ap()
    Bm = nc.dram_tensor("Bm", [MT, 128, KT, 128], bf16, kind="ExternalInput").ap()
    a = nc.dram_tensor("a", [128, MT], f32, kind="ExternalInput").ap()
    out = nc.dram_tensor("out", [MT, 128, T_CHUNK], bf16, kind="ExternalOutput").ap()

    with tile.TileContext(nc) as tc, ExitStack() as ctx:
        const = ctx.enter_context(tc.tile_pool(name="const", bufs=1))
        xt_pool = ctx.enter_context(tc.tile_pool(name="xt", bufs=NP))
        b_pool = ctx.enter_context(tc.tile_pool(name="bm", bufs=MT))
        ht_pool = ctx.enter_context(tc.tile_pool(name="ht", bufs=6))
        ps_gemm = ctx.enter_context(tc.tile_pool(name="psg", bufs=8, space="PSUM"))

        rings = [nc.sync, nc.scalar]

        bms = {}
        xps = [None] * NP

        def load_xp(p, ring):
            t = xt_pool.tile([128, 2 * T_SPAN], bf16, tag="xt", name=f"xp{p}")
            ring.dma_start(
                out=t[:].rearrange("p (k t) -> p k t", k=2),
                in_=xT[:, 2 * p : 2 * p + 2, :],
            )
            xps[p] = t

        def load_bm(m, ring, lo=0, hi=KT):
            if m not in bms:
                bms[m] = b_pool.tile([128, KT * 128], bf16, tag="bm", name=f"bm{m}")
            ring.dma_start(
                out=bms[m][:, lo * 128 : hi * 128].rearrange(
                    "p (k c) -> p k c", k=hi - lo
                ),
                in_=Bm[m, :, lo:hi, :],
            )

        # Supply plan in PE-consumption order across both rings (A=sync,
        # B=scalar).  Phase-1 needs xp0-3 + the k0-7 halves of bm0-2
        # first; the k8-15 halves and bm3+ stream behind.
        a_sb = const.tile([128, MT], f32)
        load_bm(0, nc.sync, 0, 8)       # A: bm0a
        nc.scalar.dma_start(out=a_sb, in_=a)  # B: a (tiny)
        load_xp(0, nc.scalar)           # B: xp0
        load_xp(1, nc.sync)             # A: xp1
        load_xp(2, nc.scalar)           # B: xp2
        load_xp(3, nc.sync)             # A: xp3
        load_bm(2, nc.scalar, 0, 8)     # B: bm2a
        load_bm(1, nc.sync, 0, 8)       # A: bm1a
        load_xp(4, nc.scalar)           # B: xp4
        load_xp(5, nc.sync)             # A: xp5
        load_xp(6, nc.scalar)           # B: xp6
        load_bm(0, nc.sync, 8, 16)      # A: bm0b
        load_bm(1, nc.scalar, 8, 16)    # B: bm1b
        load_xp(7, nc.sync)             # A: xp7
        load_bm(2, nc.sync, 8, 16)      # A: bm2b
        load_bm(3, nc.sync)             # A
        load_bm(4, nc.scalar)           # B
        load_bm(5, nc.sync)             # A
        load_bm(6, nc.scalar)           # B

        def xt_slice(k, lo, hi):
            return xps[k // 2][:, (k % 2) * T_SPAN + lo : (k % 2) * T_SPAN + hi]

        def emit_mm(ps, m, k, lo, hi):
            nc.tensor.matmul(
                ps[:],
                bms[m][:, k * 128 : (k + 1) * 128],
                xt_slice(k, lo, hi),
                start=(k == 0),
                stop=(k == KT - 1),
            )

        def emit_scan_out(m, psA, psB):
            ht = ht_pool.tile([128, T_SPAN], bf16, tag="ht", name=f"ht{m}")
            a_bc = a_sb[:, m : m + 1].broadcast_to([128, HALF])
            nc.vector.tensor_tensor_scan(
                ht[:, 0:HALF], a_bc, psA[:], 0.0,
                mybir.AluOpType.mult, mybir.AluOpType.add,
            )
            if m < MT - 1:
                nc.vector.tensor_tensor_scan(
                    ht[:, HALF:T_SPAN], a_bc, psB[:], ht[:, HALF - 1 : HALF],
                    mybir.AluOpType.mult, mybir.AluOpType.add,
                )
                rings[m % 2].dma_start(
                    out=out[m, :, :], in_=ht[:, W:T_SPAN]
                )
            else:
                # last m-tile: store the first half as soon as its scan is
                # done and split the trailing scan+store so the final
                # dependency chain after the last matmul is short
                rings[m % 2].dma_start(
                    out=out[m, :, 0 : HALF - W], in_=ht[:, W:HALF]
                )
                q3 = HALF + HALF // 2
                a_bc_h = a_sb[:, m : m + 1].broadcast_to([128, HALF // 2])
                nc.vector.tensor_tensor_scan(
                    ht[:, HALF:q3], a_bc_h, psB[:, 0 : HALF // 2],
                    ht[:, HALF - 1 : HALF],
                    mybir.AluOpType.mult, mybir.AluOpType.add,
                )
                rings[m % 2].dma_start(
                    out=out[m, :, HALF - W : q3 - W], in_=ht[:, HALF:q3]
                )
                nc.vector.tensor_tensor_scan(
                    ht[:, q3:T_SPAN], a_bc_h, psB[:, HALF // 2 : HALF],
                    ht[:, q3 - 1 : q3],
                    mybir.AluOpType.mult, mybir.AluOpType.add,
                )
                rings[(m + 1) % 2].dma_start(
                    out=out[m, :, q3 - W : T_CHUNK], in_=ht[:, q3:T_SPAN]
                )

        # PSUM tiles for phase-1 (A/B halves per m; accumulation groups
        # stay open across the k-half blocks).
        ph1 = {}
        for m in range(PH1):
            ph1[m] = (
                ps_gemm.tile([128, HALF], f32, tag="ps", name=f"psA{m}"),
                ps_gemm.tile([128, HALF], f32, tag="ps", name=f"psB{m}"),
            )

        # Up-front HAM warm-up: filler matmuls on a memset bf16 tile with
        # no DMA dependency, targeting psA0 — m0k0's start=True resets it.
        warm = const.tile([128, HALF], bf16)
        nc.gpsimd.memset(warm, 0.0)
        for _ in range(N_WARM):
            nc.tensor.matmul(
                ph1[0][0][:], warm[:, 0:128], warm[:], start=True, stop=True
            )

        # Phase 1: m0-m2 in k-half blocks following the x pieces.
        for m in range(PH1):
            for k in range(8):
                emit_mm(ph1[m][0], m, k, 0, HALF)
                emit_mm(ph1[m][1], m, k, HALF, T_SPAN)
        for m in range(PH1):
            for k in range(8, KT):
                emit_mm(ph1[m][0], m, k, 0, HALF)
                emit_mm(ph1[m][1], m, k, HALF, T_SPAN)
        for m in range(PH1):
            emit_scan_out(m, *ph1[m])

        # Phase 2: remaining m-tiles run dense, k-inner; B tiles stream
        # three m ahead, alternating rings.
        for m in range(PH1, MT):
            if m + 4 < MT:
                load_bm(m + 4, rings[(m + 4) % 2])
            psA = ps_gemm.tile([128, HALF], f32, tag="ps", name=f"psA{m}")
            psB = ps_gemm.tile([128, HALF], f32, tag="ps", name=f"psB{m}")
            for k in range(KT):
                emit_mm(psA, m, k, 0, HALF)
            for k in range(KT):
                emit_mm(psB, m, k, HALF, T_SPAN)
            emit_scan_out(m, psA, psB)

    nc.compile()
    return nc


def _get_nc():
    if "nc" not in _CACHE:
        _CACHE["nc"] = _build()
    return _CACHE["nc"]


def _shard_inputs(x, a, B):
    import ml_dtypes

    bf16 = ml_dtypes.bfloat16
    x = np.ascontiguousarray(x, dtype=np.float32)
    a = np.ascontiguousarray(a, dtype=np.float32)
    B = np.ascontiguousarray(B, dtype=np.float32)
    B_lin = np.ascontiguousarray(
        B.reshape(KT, 128, MT, 128).transpose(2, 1, 0, 3).astype(bf16)
    )  # [m, p, k, c] = B[128k+p, 128m+c]
    a_lin = np.ascontiguousarray(a.reshape(MT, 128).T)  # [p, m] = a[128m+p]
    xp = np.concatenate([np.zeros((W, H), np.float32), x], axis=0).astype(bf16)
    in_maps = []
    for c in range(N_CORES):
        chunk = xp[c * T_CHUNK : c * T_CHUNK + T_SPAN]  # (T_SPAN, H)
        xT_lin = np.ascontiguousarray(
            chunk.T.reshape(KT, 128, T_SPAN).transpose(1, 0, 2)
        )  # [p, k, t] = x[t, 128k+p]
        in_maps.append({"xT": xT_lin, "Bm": B_lin, "a": a_lin})
    return in_maps


def _gather_output(results):
    out = np.empty((T_FULL, H), np.float32)
    for c in range(N_CORES):
        o = np.asarray(results[c]["out"], dtype=np.float32)  # (MT, 128, T_CHUNK)
        out[c * T_CHUNK : (c + 1) * T_CHUNK] = o.reshape(H, T_CHUNK).T
    return out[None]


def _run(inputs, trace=False):
    from concourse import bass_utils

    nc = _get_nc()
    in_maps = _shard_inputs(inputs["x"], inputs["a"], inputs["B"])
    res = bass_utils.run_bass_kernel_spmd(
        nc, in_maps, core_ids=list(range(N_CORES)), trace=trace
    )
    return _gather_output(res.results), res


def kernel(x, a, B):
    out, _ = _run({"x": x, "a": a, "B": B})
    return out
